# revision 17
# baseline (speedup 1.0000x reference)
"""Trainium2 Bass kernel for nn_BoundaryLoss (boundary EDT + weighted L1 loss).

Strategy (pure data parallel, 1 image per NeuronCore, 8 cores):
  Per image on device:
    binary  = target > 0.5
    bound   = binary - erode3x3(binary)          (via complement dilation)
    d2      = windowed exact Euclidean distance transform of bound
              phase 1: vertical L1 distance via log-doubling (window 7)
              phase 2: horizontal parabola min over offsets |u| <= 4
    outputs per partition: sum(sqrt(d2) * |sigmoid(pred)-target|), max(d2)
  Host: final 256-way reduction per image + normalization + batch mean.

Windowed EDT exactness: windowed d2 >= true d2 always, with equality
guaranteed when max(windowed d2) <= K^2 (K = 4): any closer out-of-window
feature would have |di|,|dj| < K and hence be in-window.  The device
returns max(d2); the host verifies the bound and falls back to an exact
numpy path for any image that fails it (never on dense masks).

Raw bass (no Tile): the pipeline is linear across 4 engines (DVE chain,
ACT helper ops, PE transposes, SP DMA), so stage-boundary semaphores are
enough, every instruction carries <= 2 sync waits (ISA limit), and there
is no Tile kernel-tail barrier overhead.

All inputs ship as ONE DRAM tensor (target rows 0:256, pred rows 256:512,
an f32 identity block rows 512:640) so a single input DMA feeds the core.
"""

import os
from contextlib import ExitStack

import numpy as np

H = 256
W = 256
P = 128
C = 2  # partition chunks per image (H = C * P)
KH = 4  # phase-2 horizontal window (exactness proof bound: m2 <= KH*KH)
BIGF = 16384.0  # phase-1 sentinel (bf16-exact; BIGF + small stays BIGF in bf16)
BIG2 = 3.0e8  # phase-2 border pad, > BIGF^2
PAD1 = 7  # phase-1 doubling pads (window 1+2+4)
FW = H + 2 * PAD1
GW = W + 2 * KH

LAST_RESULTS = None  # BassKernelResults of the most recent device run


def _build_nc():
    import concourse.bass as bass
    import concourse.mybir as mybir

    bf16 = mybir.dt.bfloat16
    f32 = mybir.dt.float32
    Alu = mybir.AluOpType
    Act = mybir.ActivationFunctionType

    nc = bass.Bass()
    inp_d = nc.dram_tensor("inp", [5 * P, W], f32, kind="ExternalInput")
    out_d = nc.dram_tensor("out", [P, 4], f32, kind="ExternalOutput")

    ctx = ExitStack()
    sb = lambda name, shape, dt: ctx.enter_context(nc.sbuf_tensor(name, shape, dt))
    ps = lambda name: ctx.enter_context(nc.psum_tensor(name, [P, P], bf16))

    with ctx:
        inp = sb("inp_t", [P, 5, W], f32)
        tgt = inp[:, 0:C, :]
        prd = inp[:, C : 2 * C, :]
        ident = sb("ident", [P, P], bf16)
        nbp = sb("nbp", [P, C, W + 2], bf16)
        b_t = sb("b_t", [P, C, W], bf16)
        t1 = sb("t1", [P, C, W], bf16)
        dr = sb("dr", [P, C, W], bf16)
        bT = sb("bT", [P, C, H], bf16)
        drTp = sb("drTp", [P, C, H + 2], bf16)
        t2 = sb("t2", [P, C, H], bf16)
        dT = sb("dT", [P, C, H], bf16)
        boundT = sb("boundT", [P, C, H], bf16)
        fvA = sb("fvA", [P, C, FW], bf16)
        fvB = sb("fvB", [P, C, FW], bf16)
        tmpd = sb("tmpd", [P, C, FW], bf16)
        g2T = sb("g2T", [P, C, H], bf16)
        g2p = sb("g2p", [P, C, GW], bf16)
        p2tmp = sb("p2tmp", [P, C, W], bf16)
        p2acc = [sb(f"p2acc{i}", [P, C, W], bf16) for i in range(KH)]
        dist = sb("dist", [P, C, W], f32)
        sg = sb("sg", [P, C, W], f32)
        diff = sb("diff", [P, C, W], f32)
        adiff = sb("adiff", [P, C, W], f32)
        junk = sb("junk", [P, C, W], f32)
        outb = sb("outb", [P, 4], f32)
        blks = [ps(f"blk{i}") for i in range(8)]

        dma_sem = ctx.enter_context(nc.semaphore("dma_sem"))
        dve_sem = ctx.enter_context(nc.semaphore("dve_sem"))
        act_sem = ctx.enter_context(nc.semaphore("act_sem"))
        pe_sem = ctx.enter_context(nc.semaphore("pe_sem"))

        block = ctx.enter_context(nc.Block())

        @block.sync
        def _(sync: "bass.BassEngine"):
            sync.dma_start(out=inp[:], in_=inp_d.rearrange("(a p) w -> p a w", p=P)).then_inc(dma_sem, 16)
            # out DMA (after the DVE chain fully wrote outb)
            sync.wait_ge(dve_sem, 4)
            sync.dma_start(out=out_d[:], in_=outb[:]).then_inc(dma_sem, 16)
            sync.wait_ge(dma_sem, 32)

        @block.scalar
        def _(scalar: "bass.BassEngine"):
            # ident convert + sigmoid: only need the input DMA
            scalar.wait_ge(dma_sem, 16)
            nc.scalar.copy(ident[:], inp[:, 2 * C, 0:P])
            nc.scalar.activation(sg[:], prd, Act.Sigmoid).then_inc(act_sem, 1)  # a=1
            # stage A copies: 8 transpose blocks (b, dr) -> bT, drTp
            k = 0
            for dst, ofs in ((bT, 0), (drTp, 1)):
                for wb in range(C):
                    for hc in range(C):
                        scalar.wait_ge(pe_sem, k + 1)
                        ins = nc.scalar.copy(
                            dst[:, wb, ofs + hc * P : ofs + (hc + 1) * P], blks[k][:]
                        )
                        k += 1
            ins.then_inc(act_sem, 1)  # a=2
            # square of vertical L1 distance (after DVE doubling: d=2)
            scalar.wait_ge(dve_sem, 2)
            nc.scalar.square(g2T[:], fvB[:, :, PAD1 : PAD1 + H]).then_inc(act_sem, 1)  # a=3
            # stage B copies: 4 transpose blocks g2T -> g2p
            for k in range(4):
                scalar.wait_ge(pe_sem, 9 + k)
                wb, hc = divmod(k, C)
                ins = nc.scalar.copy(g2p[:, hc, KH + wb * P : KH + (wb + 1) * P], blks[k][:])
            ins.then_inc(act_sem, 1)  # a=4
            # sqrt(d2) (after DVE phase 2: d=3)
            scalar.wait_ge(dve_sem, 3)
            nc.scalar.sqrt(dist[:], p2acc[KH - 1][:]).then_inc(act_sem, 1)  # a=5

        @block.tensor
        def _(tensor: "bass.BassEngine"):
            # stage A transposes: b, dr (DVE d=1) using ident (ACT a=1)
            tensor.wait_ge(act_sem, 1)
            tensor.wait_ge(dve_sem, 1)
            k = 0
            for src in (b_t, dr):
                for wb in range(C):
                    for hc in range(C):
                        nc.tensor.transpose(
                            blks[k][:], src[:, hc, wb * P : (wb + 1) * P], ident[:]
                        ).then_inc(pe_sem, 1)
                        k += 1
            # stage B transposes: g2T (ACT a=3); blks 0-3 reuse needs ACT a=2 (done, a=3 > 2)
            tensor.wait_ge(act_sem, 3)
            nc.tensor.drain()
            for k in range(4):
                wb, hc = divmod(k, C)
                nc.tensor.transpose(
                    blks[k][:], g2T[:, wb, hc * P : (hc + 1) * P], ident[:]
                ).then_inc(pe_sem, 1)

        @block.vector
        def _(vector: "bass.BassEngine"):
            D = nc.vector.drain  # same-engine RAW ordering (DVE pipe flush)
            # data-independent pad memsets first (no waits)
            nc.vector.memset(nbp[:, :, 0:1], 0.0)
            nc.vector.memset(nbp[:, :, W + 1 : W + 2], 0.0)
            nc.vector.memset(drTp[:, :, 0:1], 0.0)
            nc.vector.memset(drTp[:, :, H + 1 : H + 2], 0.0)
            nc.vector.memset(fvA[:, :, 0:PAD1], BIGF)
            nc.vector.memset(fvA[:, :, PAD1 + H : FW], BIGF)
            nc.vector.memset(fvB[:, :, 0:1], BIGF)
            nc.vector.memset(fvB[:, :, FW - 1 : FW], BIGF)
            nc.vector.memset(g2p[:, :, 0:KH], BIG2)
            nc.vector.memset(g2p[:, :, KH + W : GW], BIG2)
            nc.vector.memset(outb[:, 3:4], 0.0)
            D()

            vector.wait_ge(dma_sem, 16)
            nc.vector.tensor_scalar(nbp[:, :, 1 : W + 1], tgt, 0.5, None, Alu.is_le)
            nc.vector.tensor_scalar(b_t[:], tgt, 0.5, None, Alu.is_gt)
            D()
            # horizontal dilation of complement
            nc.vector.tensor_tensor(t1[:], nbp[:, :, 0:W], nbp[:, :, 2 : W + 2], Alu.max)
            D()
            nc.vector.tensor_tensor(dr[:], t1[:], nbp[:, :, 1 : W + 1], Alu.max)
            D().then_inc(dve_sem, 1)  # d=1

            # vertical dilation + boundaries (needs ACT stage A copies: a=2)
            vector.wait_ge(act_sem, 2)
            nc.vector.tensor_tensor(t2[:], drTp[:, :, 0:H], drTp[:, :, 2 : H + 2], Alu.max)
            D()
            nc.vector.tensor_tensor(dT[:], t2[:], drTp[:, :, 1 : H + 1], Alu.max)
            D()
            nc.vector.tensor_tensor(boundT[:], bT[:], dT[:], Alu.min)
            D()
            nc.vector.tensor_scalar(
                fvA[:, :, PAD1 : PAD1 + H], boundT[:], -BIGF, BIGF, Alu.mult, Alu.add
            )
            D()
            # vertical L1 distance by log-doubling (window 1+2+4 = 7)
            cur, nxt = fvA, fvB
            for d in (1, 2, 4):
                lo, hi = d, FW - d
                nc.vector.tensor_tensor(
                    tmpd[:, :, lo:hi], cur[:, :, 0 : FW - 2 * d], cur[:, :, 2 * d : FW], Alu.min
                )
                D()
                nc.vector.scalar_tensor_tensor(
                    out=nxt[:, :, lo:hi],
                    in0=tmpd[:, :, lo:hi],
                    scalar=float(d),
                    in1=cur[:, :, lo:hi],
                    op0=Alu.add,
                    op1=Alu.min,
                )
                ins = D()
                cur, nxt = nxt, cur
            ins.then_inc(dve_sem, 1)  # d=2 (fvB = vertical L1 dist)

            # phase 2 (needs ACT stage B copies: a=4)
            vector.wait_ge(act_sem, 4)
            prev = None
            for u in range(1, KH + 1):
                nc.vector.tensor_tensor(
                    p2tmp[:], g2p[:, :, KH - u : KH - u + W], g2p[:, :, KH + u : KH + u + W], Alu.min
                )
                D()
                base = g2p[:, :, KH : KH + W] if prev is None else prev[:]
                nc.vector.scalar_tensor_tensor(
                    out=p2acc[u - 1][:], in0=p2tmp[:], scalar=float(u * u), in1=base,
                    op0=Alu.add, op1=Alu.min,
                )
                ins = D()
                prev = p2acc[u - 1]
            d2 = prev
            ins.then_inc(dve_sem, 1)  # d=3 (d2 ready for ACT sqrt)
            nc.vector.tensor_reduce(
                out=outb[:, 1:3], in_=d2[:], axis=mybir.AxisListType.X, op=Alu.max
            )
            D()
            # weighted L1 (needs sg: a>=1 [covered by a>=4], dist: a=5)
            nc.vector.tensor_tensor(diff[:], sg[:], tgt, Alu.subtract)
            D()
            nc.vector.tensor_scalar(junk[:], diff[:], -1.0, None, Alu.mult)
            D()
            nc.vector.tensor_tensor(adiff[:], diff[:], junk[:], Alu.max)
            D()
            vector.wait_ge(act_sem, 5)
            nc.vector.tensor_tensor(junk[:], dist[:], adiff[:], Alu.mult)
            D()
            nc.vector.tensor_reduce(
                out=outb[:, 0:1], in_=junk[:], axis=mybir.AxisListType.XY, op=Alu.add
            )
            D().then_inc(dve_sem, 1)  # d=4 (outb complete)

    return nc


_NC_CACHE = {}


def _get_nc():
    if "nc" not in _NC_CACHE:
        _NC_CACHE["nc"] = _build_nc()
    return _NC_CACHE["nc"]


def _pack_input(tgt_i, prd_i, ident_block):
    return np.concatenate([tgt_i, prd_i, ident_block], axis=0)


# ---------- exact numpy fallback (pathological images only) ----------

def _reference_image_np(t, p):
    """Exact replica of the jax reference for one image, in numpy fp32."""
    b = (t > 0.5).astype(np.float32)
    if not (b > 0).any():
        return 0.0
    # erode3x3 with +inf border
    v = b.copy()
    v[1:] = np.minimum(v[1:], b[:-1])
    v[:-1] = np.minimum(v[:-1], b[1:])
    er = v.copy()
    er[:, 1:] = np.minimum(er[:, 1:], v[:, :-1])
    er[:, :-1] = np.minimum(er[:, :-1], v[:, 1:])
    bound = b - er
    if bound.sum() == 0:
        bound = b
    feat = bound > 0.5
    BIGV = np.float32(1e6)
    c = np.full(W, BIGV, np.float32)
    d_fwd = np.empty((H, W), np.float32)
    for i in range(H):
        c = np.where(feat[i], np.float32(0.0), c + 1)
        d_fwd[i] = c
    c = np.full(W, BIGV, np.float32)
    d_bwd = np.empty((H, W), np.float32)
    for i in range(H - 1, -1, -1):
        c = np.where(feat[i], np.float32(0.0), c + 1)
        d_bwd[i] = c
    g = np.minimum(d_fwd, d_bwd)
    j = np.arange(W, dtype=np.float32)
    d2 = np.empty((H, W), np.float32)
    for i in range(H):
        d2[i] = np.min(g[i][None, :] ** 2 + (j[:, None] - j[None, :]) ** 2, axis=-1)
    dist = np.sqrt(d2)
    m = dist.max()
    if m > 0:
        dist = dist / (m + np.float32(1e-8))
    sgm = 1.0 / (1.0 + np.exp(-p.astype(np.float64)))
    return float(np.mean(dist * np.abs(sgm - t)))


def _bound_empty(t):
    """True if erosion removes every boundary pixel (reference falls back)."""
    b = (t > 0.5).astype(np.float32)
    v = b.copy()
    v[1:] = np.minimum(v[1:], b[:-1])
    v[:-1] = np.minimum(v[:-1], b[1:])
    er = v.copy()
    er[:, 1:] = np.minimum(er[:, 1:], v[:, :-1])
    er[:, :-1] = np.minimum(er[:, :-1], v[:, 1:])
    return (b - er).sum() == 0


# ---------- public entry point ----------

def kernel(pred_logits: np.ndarray, target: np.ndarray) -> np.ndarray:
    global LAST_RESULTS
    from concourse.bass_utils import run_bass_kernel_spmd

    pred = np.ascontiguousarray(np.asarray(pred_logits, np.float32)[:, 0])
    tgt = np.ascontiguousarray(np.asarray(target, np.float32)[:, 0])
    B = pred.shape[0]
    assert pred.shape == (B, H, W) and tgt.shape == (B, H, W)
    assert B == 8, f"kernel is built for batch 8, got {B}"

    ident_block = np.zeros((P, W), np.float32)
    ident_block[:, :P] = np.eye(P, dtype=np.float32)

    nc = _get_nc()
    in_maps = [{"inp": _pack_input(tgt[i], pred[i], ident_block)} for i in range(B)]
    trace = bool(int(os.environ.get("KERNEL_TRACE", "0")))
    res = run_bass_kernel_spmd(nc, in_maps, core_ids=list(range(B)), trace=trace)
    LAST_RESULTS = res

    total = 0.0
    for i in range(B):
        o = np.asarray(res.results[i]["out"], np.float32)  # [128, 4]
        if not (tgt[i] > 0.5).any():
            continue  # empty mask: reference skips (loss 0)
        m2 = float(o[:, 1:3].max())
        if m2 > float(KH * KH) or _bound_empty(tgt[i]):
            # windowed EDT not provably exact for this image -> exact path
            total += _reference_image_np(tgt[i], pred[i])
            continue
        S = float(o[:, 0].sum(dtype=np.float64))
        m = np.float32(np.sqrt(np.float32(m2)))
        denom = float(m + np.float32(1e-8)) if m > 0 else 1.0
        total += (S / denom) / float(H * W)
    return np.float32(total / max(B, 1))


# revision 19
# speedup vs baseline: 1.0813x; 1.0813x over previous
"""Trainium2 Bass kernel for nn_BoundaryLoss (boundary EDT + weighted L1 loss).

Strategy (pure data parallel, 1 image per NeuronCore, 8 cores):
  Per image on device:
    binary  = target > 0.5
    bound   = binary - erode3x3(binary)          (via complement dilation)
    d2      = windowed exact Euclidean distance transform of bound
              phase 1: vertical L1 distance via log-doubling (window 7)
              phase 2: horizontal parabola min over offsets |u| <= 4
    outputs per partition: sum(sqrt(d2) * |sigmoid(pred)-target|), max(d2)
  Host: final 256-way reduction per image + normalization + batch mean.

Windowed EDT exactness: windowed d2 >= true d2 always, with equality
guaranteed when max(windowed d2) <= K^2 (K = 4): any closer out-of-window
feature would have |di|,|dj| < K and hence be in-window.  The device
returns max(d2); the host verifies the bound and falls back to an exact
numpy path for any image that fails it (never on dense masks).

Raw bass (no Tile): the pipeline is linear across 4 engines (DVE chain,
ACT helper ops, PE transposes, SP DMA), so stage-boundary semaphores are
enough, every instruction carries <= 2 sync waits (ISA limit), and there
is no Tile kernel-tail barrier overhead.

All inputs ship as ONE DRAM tensor (target rows 0:256, pred rows 256:512,
an f32 identity block rows 512:640) so a single input DMA feeds the core.
"""

import os
from contextlib import ExitStack

import numpy as np

H = 256
W = 256
P = 128
C = 2  # partition chunks per image (H = C * P)
KH = 4  # phase-2 horizontal window (exactness proof bound: m2 <= KH*KH)
BIGF = 16384.0  # phase-1 sentinel (bf16-exact; BIGF + small stays BIGF in bf16)
BIG2 = 3.0e8  # phase-2 border pad, > BIGF^2
PAD1 = 7  # phase-1 doubling pads (window 1+2+4)
FW = H + 2 * PAD1
GW = W + 2 * KH

LAST_RESULTS = None  # BassKernelResults of the most recent device run


def _build_nc():
    import concourse.bass as bass
    import concourse.mybir as mybir

    bf16 = mybir.dt.bfloat16
    f32 = mybir.dt.float32
    Alu = mybir.AluOpType
    Act = mybir.ActivationFunctionType

    nc = bass.Bass(detect_race_conditions=False)
    inp_d = nc.dram_tensor("inp", [5 * P, W], f32, kind="ExternalInput")
    out_d = nc.dram_tensor("out", [P, 4], f32, kind="ExternalOutput")

    ctx = ExitStack()
    sb = lambda name, shape, dt: ctx.enter_context(nc.sbuf_tensor(name, shape, dt))
    ps = lambda name: ctx.enter_context(nc.psum_tensor(name, [P, P], bf16))

    with ctx:
        inp = sb("inp_t", [P, 5, W], f32)
        tgt = inp[:, 0:C, :]
        prd = inp[:, C : 2 * C, :]
        ident = sb("ident", [P, P], bf16)
        nbp = sb("nbp", [P, C, W + 2], bf16)
        b_t = sb("b_t", [P, C, W], bf16)
        t1 = sb("t1", [P, C, W], bf16)
        dr = sb("dr", [P, C, W], bf16)
        bT = sb("bT", [P, C, H], bf16)
        drTp = sb("drTp", [P, C, H + 2], bf16)
        t2 = sb("t2", [P, C, H], bf16)
        dT = sb("dT", [P, C, H], bf16)
        boundT = sb("boundT", [P, C, H], bf16)
        fvA = sb("fvA", [P, C, FW], bf16)
        fvB = sb("fvB", [P, C, FW], bf16)
        tmpd = sb("tmpd", [P, C, FW], bf16)
        g2T = sb("g2T", [P, C, H], bf16)
        g2p = sb("g2p", [P, C, GW], bf16)
        p2tmp = sb("p2tmp", [P, C, W], bf16)
        p2acc = [sb(f"p2acc{i}", [P, C, W], bf16) for i in range(KH)]
        dist = sb("dist", [P, C, W], f32)
        sg = sb("sg", [P, C, W], f32)
        diff = sb("diff", [P, C, W], f32)
        adiff = sb("adiff", [P, C, W], f32)
        junk = sb("junk", [P, C, W], f32)
        outb = sb("outb", [P, 4], f32)
        blks = [ps(f"blk{i}") for i in range(8)]

        dma_sem = ctx.enter_context(nc.semaphore("dma_sem"))
        dve_sem = ctx.enter_context(nc.semaphore("dve_sem"))
        act_sem = ctx.enter_context(nc.semaphore("act_sem"))
        pe_sem = ctx.enter_context(nc.semaphore("pe_sem"))

        block = ctx.enter_context(nc.Block(no_gpsimd_drain=True))

        @block.sync
        def _(sync: "bass.BassEngine"):
            sync.dma_start(out=inp[:], in_=inp_d.rearrange("(a p) w -> p a w", p=P)).then_inc(dma_sem, 16)
            # out DMA (after the DVE chain fully wrote outb)
            sync.wait_ge(dve_sem, 4)
            sync.dma_start(out=out_d[:], in_=outb[:]).then_inc(dma_sem, 16)
            sync.wait_ge(dma_sem, 32)

        @block.scalar
        def _(scalar: "bass.BassEngine"):
            # ident convert + sigmoid: only need the input DMA
            scalar.wait_ge(dma_sem, 16)
            nc.scalar.copy(ident[:], inp[:, 2 * C, 0:P])
            nc.scalar.activation(sg[:], prd, Act.Sigmoid).then_inc(act_sem, 1)  # a=1
            # warm the Square/Sqrt activation tables off the critical path
            nc.scalar.square(dist[:, 0, 0:1], inp[:, 0, 0:1])
            nc.scalar.sqrt(dist[:, 0, 0:1], inp[:, 0, 0:1])
            # stage A copies: 8 transpose blocks (b, dr) -> bT, drTp
            k = 0
            for dst, ofs in ((bT, 0), (drTp, 1)):
                for wb in range(C):
                    for hc in range(C):
                        scalar.wait_ge(pe_sem, k + 1)
                        ins = nc.scalar.copy(
                            dst[:, wb, ofs + hc * P : ofs + (hc + 1) * P], blks[k][:]
                        )
                        k += 1
            ins.then_inc(act_sem, 1)  # a=2
            # square of vertical L1 distance (after DVE doubling: d=2)
            scalar.wait_ge(dve_sem, 2)
            nc.scalar.square(g2T[:], fvB[:, :, PAD1 : PAD1 + H]).then_inc(act_sem, 1)  # a=3
            # stage B copies: 4 transpose blocks g2T -> g2p
            for k in range(4):
                scalar.wait_ge(pe_sem, 9 + k)
                wb, hc = divmod(k, C)
                ins = nc.scalar.copy(g2p[:, hc, KH + wb * P : KH + (wb + 1) * P], blks[k][:])
            ins.then_inc(act_sem, 1)  # a=4
            # sqrt(d2) (after DVE phase 2: d=3)
            scalar.wait_ge(dve_sem, 3)
            nc.scalar.sqrt(dist[:], p2acc[KH - 1][:]).then_inc(act_sem, 1)  # a=5

        @block.tensor
        def _(tensor: "bass.BassEngine"):
            # stage A transposes: b, dr (DVE d=1) using ident (ACT a=1)
            tensor.wait_ge(act_sem, 1)
            tensor.wait_ge(dve_sem, 1)
            k = 0
            for src in (b_t, dr):
                for wb in range(C):
                    for hc in range(C):
                        nc.tensor.transpose(
                            blks[k][:], src[:, hc, wb * P : (wb + 1) * P], ident[:]
                        ).then_inc(pe_sem, 1)
                        k += 1
            # stage B transposes: g2T (ACT a=3); blks 0-3 reuse needs ACT a=2 (done, a=3 > 2)
            tensor.wait_ge(act_sem, 3)
            for k in range(4):
                wb, hc = divmod(k, C)
                nc.tensor.transpose(
                    blks[k][:], g2T[:, wb, hc * P : (hc + 1) * P], ident[:]
                ).then_inc(pe_sem, 1)

        @block.vector
        def _(vector: "bass.BassEngine"):
            # data-independent pad memsets first (no waits)
            nc.vector.memset(nbp[:, :, 0:1], 0.0)
            nc.vector.memset(nbp[:, :, W + 1 : W + 2], 0.0)
            nc.vector.memset(drTp[:, :, 0:1], 0.0)
            nc.vector.memset(drTp[:, :, H + 1 : H + 2], 0.0)
            nc.vector.memset(fvA[:, :, 0:PAD1], BIGF)
            nc.vector.memset(fvA[:, :, PAD1 + H : FW], BIGF)
            nc.vector.memset(fvB[:, :, 0:1], BIGF)
            nc.vector.memset(fvB[:, :, FW - 1 : FW], BIGF)
            nc.vector.memset(g2p[:, :, 0:KH], BIG2)
            nc.vector.memset(g2p[:, :, KH + W : GW], BIG2)
            nc.vector.memset(outb[:, 3:4], 0.0)

            vector.wait_ge(dma_sem, 16)
            nc.vector.tensor_scalar(nbp[:, :, 1 : W + 1], tgt, 0.5, None, Alu.is_le)
            nc.vector.tensor_scalar(b_t[:], tgt, 0.5, None, Alu.is_gt)
            # horizontal dilation of complement
            nc.vector.tensor_tensor(t1[:], nbp[:, :, 0:W], nbp[:, :, 2 : W + 2], Alu.max)
            nc.vector.tensor_tensor(dr[:], t1[:], nbp[:, :, 1 : W + 1], Alu.max).then_inc(dve_sem, 1)  # d=1

            # vertical dilation + boundaries (needs ACT stage A copies: a=2)
            vector.wait_ge(act_sem, 2)
            nc.vector.tensor_tensor(t2[:], drTp[:, :, 0:H], drTp[:, :, 2 : H + 2], Alu.max)
            nc.vector.tensor_tensor(dT[:], t2[:], drTp[:, :, 1 : H + 1], Alu.max)
            nc.vector.tensor_tensor(boundT[:], bT[:], dT[:], Alu.min)
            nc.vector.tensor_scalar(
                fvA[:, :, PAD1 : PAD1 + H], boundT[:], -BIGF, BIGF, Alu.mult, Alu.add
            )
            # vertical L1 distance by log-doubling (window 1+2+4 = 7)
            cur, nxt = fvA, fvB
            for d in (1, 2, 4):
                lo, hi = d, FW - d
                nc.vector.tensor_tensor(
                    tmpd[:, :, lo:hi], cur[:, :, 0 : FW - 2 * d], cur[:, :, 2 * d : FW], Alu.min
                )
                ins = nc.vector.scalar_tensor_tensor(
                    out=nxt[:, :, lo:hi],
                    in0=tmpd[:, :, lo:hi],
                    scalar=float(d),
                    in1=cur[:, :, lo:hi],
                    op0=Alu.add,
                    op1=Alu.min,
                )
                cur, nxt = nxt, cur
            ins.then_inc(dve_sem, 1)  # d=2 (fvB = vertical L1 dist)

            # phase 2 (needs ACT stage B copies: a=4)
            vector.wait_ge(act_sem, 4)
            prev = None
            for u in range(1, KH + 1):
                nc.vector.tensor_tensor(
                    p2tmp[:], g2p[:, :, KH - u : KH - u + W], g2p[:, :, KH + u : KH + u + W], Alu.min
                )
                base = g2p[:, :, KH : KH + W] if prev is None else prev[:]
                ins = nc.vector.scalar_tensor_tensor(
                    out=p2acc[u - 1][:], in0=p2tmp[:], scalar=float(u * u), in1=base,
                    op0=Alu.add, op1=Alu.min,
                )
                prev = p2acc[u - 1]
            d2 = prev
            ins.then_inc(dve_sem, 1)  # d=3 (d2 ready for ACT sqrt)
            nc.vector.tensor_reduce(
                out=outb[:, 1:3], in_=d2[:], axis=mybir.AxisListType.X, op=Alu.max
            )
            # weighted L1 (needs sg: a>=1 [covered by a>=4], dist: a=5)
            nc.vector.tensor_tensor(diff[:], sg[:], tgt, Alu.subtract)
            nc.vector.tensor_scalar(junk[:], diff[:], -1.0, None, Alu.mult)
            nc.vector.tensor_tensor(adiff[:], diff[:], junk[:], Alu.max)
            vector.wait_ge(act_sem, 5)
            nc.vector.tensor_tensor(junk[:], dist[:], adiff[:], Alu.mult)
            nc.vector.tensor_reduce(
                out=outb[:, 0:1], in_=junk[:], axis=mybir.AxisListType.XY, op=Alu.add
            ).then_inc(dve_sem, 1)  # d=4 (outb complete)

    return nc


_NC_CACHE = {}


def _get_nc():
    if "nc" not in _NC_CACHE:
        _NC_CACHE["nc"] = _build_nc()
    return _NC_CACHE["nc"]


def _pack_input(tgt_i, prd_i, ident_block):
    return np.concatenate([tgt_i, prd_i, ident_block], axis=0)


# ---------- exact numpy fallback (pathological images only) ----------

def _reference_image_np(t, p):
    """Exact replica of the jax reference for one image, in numpy fp32."""
    b = (t > 0.5).astype(np.float32)
    if not (b > 0).any():
        return 0.0
    # erode3x3 with +inf border
    v = b.copy()
    v[1:] = np.minimum(v[1:], b[:-1])
    v[:-1] = np.minimum(v[:-1], b[1:])
    er = v.copy()
    er[:, 1:] = np.minimum(er[:, 1:], v[:, :-1])
    er[:, :-1] = np.minimum(er[:, :-1], v[:, 1:])
    bound = b - er
    if bound.sum() == 0:
        bound = b
    feat = bound > 0.5
    BIGV = np.float32(1e6)
    c = np.full(W, BIGV, np.float32)
    d_fwd = np.empty((H, W), np.float32)
    for i in range(H):
        c = np.where(feat[i], np.float32(0.0), c + 1)
        d_fwd[i] = c
    c = np.full(W, BIGV, np.float32)
    d_bwd = np.empty((H, W), np.float32)
    for i in range(H - 1, -1, -1):
        c = np.where(feat[i], np.float32(0.0), c + 1)
        d_bwd[i] = c
    g = np.minimum(d_fwd, d_bwd)
    j = np.arange(W, dtype=np.float32)
    d2 = np.empty((H, W), np.float32)
    for i in range(H):
        d2[i] = np.min(g[i][None, :] ** 2 + (j[:, None] - j[None, :]) ** 2, axis=-1)
    dist = np.sqrt(d2)
    m = dist.max()
    if m > 0:
        dist = dist / (m + np.float32(1e-8))
    sgm = 1.0 / (1.0 + np.exp(-p.astype(np.float64)))
    return float(np.mean(dist * np.abs(sgm - t)))


def _bound_empty(t):
    """True if erosion removes every boundary pixel (reference falls back)."""
    b = (t > 0.5).astype(np.float32)
    v = b.copy()
    v[1:] = np.minimum(v[1:], b[:-1])
    v[:-1] = np.minimum(v[:-1], b[1:])
    er = v.copy()
    er[:, 1:] = np.minimum(er[:, 1:], v[:, :-1])
    er[:, :-1] = np.minimum(er[:, :-1], v[:, 1:])
    return (b - er).sum() == 0


# ---------- public entry point ----------

def kernel(pred_logits: np.ndarray, target: np.ndarray) -> np.ndarray:
    global LAST_RESULTS
    from concourse.bass_utils import run_bass_kernel_spmd

    pred = np.ascontiguousarray(np.asarray(pred_logits, np.float32)[:, 0])
    tgt = np.ascontiguousarray(np.asarray(target, np.float32)[:, 0])
    B = pred.shape[0]
    assert pred.shape == (B, H, W) and tgt.shape == (B, H, W)
    assert B == 8, f"kernel is built for batch 8, got {B}"

    ident_block = np.zeros((P, W), np.float32)
    ident_block[:, :P] = np.eye(P, dtype=np.float32)

    nc = _get_nc()
    in_maps = [{"inp": _pack_input(tgt[i], pred[i], ident_block)} for i in range(B)]
    trace = bool(int(os.environ.get("KERNEL_TRACE", "0")))
    res = run_bass_kernel_spmd(nc, in_maps, core_ids=list(range(B)), trace=trace)
    LAST_RESULTS = res

    total = 0.0
    for i in range(B):
        o = np.asarray(res.results[i]["out"], np.float32)  # [128, 4]
        if not (tgt[i] > 0.5).any():
            continue  # empty mask: reference skips (loss 0)
        m2 = float(o[:, 1:3].max())
        if m2 > float(KH * KH) or _bound_empty(tgt[i]):
            # windowed EDT not provably exact for this image -> exact path
            total += _reference_image_np(tgt[i], pred[i])
            continue
        S = float(o[:, 0].sum(dtype=np.float64))
        m = np.float32(np.sqrt(np.float32(m2)))
        denom = float(m + np.float32(1e-8)) if m > 0 else 1.0
        total += (S / denom) / float(H * W)
    return np.float32(total / max(B, 1))


# revision 22
# speedup vs baseline: 1.1844x; 1.0954x over previous
"""Trainium2 Bass kernel for nn_BoundaryLoss (boundary EDT + weighted L1 loss).

Strategy (pure data parallel, 1 image per NeuronCore, 8 cores):
  Per image on device:
    binary  = target > 0.5
    bound   = binary - erode3x3(binary)          (via complement dilation)
    d2      = windowed exact Euclidean distance transform of bound
              phase 1: vertical L1 distance via log-doubling (window 7)
              phase 2: horizontal parabola min over offsets |u| <= 4
    outputs per partition: sum(sqrt(d2) * |sigmoid(pred)-target|), max(d2)
  Host: final 256-way reduction per image + normalization + batch mean.

Windowed EDT exactness: windowed d2 >= true d2 always, with equality
guaranteed when max(windowed d2) <= K^2 (K = 4): any closer out-of-window
feature would have |di|,|dj| < K and hence be in-window.  The device
returns max(d2); the host verifies the bound and falls back to an exact
numpy path for any image that fails it (never on dense masks).

Raw bass (no Tile): the pipeline is linear across 4 engines (DVE chain,
ACT helper ops, PE transposes, SP DMA), so stage-boundary semaphores are
enough, every instruction carries <= 2 sync waits (ISA limit), and there
is no Tile kernel-tail barrier overhead.

All inputs ship as ONE DRAM tensor (target rows 0:256, pred rows 256:512,
an f32 identity block rows 512:640) so a single input DMA feeds the core.
"""

import os
from contextlib import ExitStack

import numpy as np

H = 256
W = 256
P = 128
C = 2  # partition chunks per image (H = C * P)
KH = 4  # phase-2 horizontal window (exactness proof bound: m2 <= KH*KH)
BIGF = 16384.0  # phase-1 sentinel (bf16-exact; BIGF + small stays BIGF in bf16)
BIG2 = 3.0e8  # phase-2 border pad, > BIGF^2
PAD1 = 7  # phase-1 doubling pads (window 1+2+4)
FW = H + 2 * PAD1
GW = W + 2 * KH

LAST_RESULTS = None  # BassKernelResults of the most recent device run


def _build_nc():
    import concourse.bass as bass
    import concourse.mybir as mybir

    bf16 = mybir.dt.bfloat16
    f32 = mybir.dt.float32
    Alu = mybir.AluOpType
    Act = mybir.ActivationFunctionType

    nc = bass.Bass(detect_race_conditions=False)
    inp_d = nc.dram_tensor("inp", [P, 5 * W], f32, kind="ExternalInput")
    out_d = nc.dram_tensor("out", [P, 4], f32, kind="ExternalOutput")

    ctx = ExitStack()
    sb = lambda name, shape, dt: ctx.enter_context(nc.sbuf_tensor(name, shape, dt))
    ps = lambda name: ctx.enter_context(nc.psum_tensor(name, [P, P], bf16))

    with ctx:
        inp = sb("inp_t", [P, 5, W], f32)
        tgt = inp[:, 0:C, :]
        prd = inp[:, C : 2 * C, :]
        ident = sb("ident", [P, P], bf16)
        nbp = sb("nbp", [P, C, W + 2], bf16)
        b_t = sb("b_t", [P, C, W], bf16)
        t1 = sb("t1", [P, C, W], bf16)
        dr = sb("dr", [P, C, W], bf16)
        bT = sb("bT", [P, C, H], bf16)
        drTp = sb("drTp", [P, C, H + 2], bf16)
        t2 = sb("t2", [P, C, H], bf16)
        dT = sb("dT", [P, C, H], bf16)
        boundT = sb("boundT", [P, C, H], bf16)
        fvA = sb("fvA", [P, C, FW], bf16)
        fvB = sb("fvB", [P, C, FW], bf16)
        tmpd = sb("tmpd", [P, C, FW], bf16)
        g2T = sb("g2T", [P, C, H], bf16)
        g2p = sb("g2p", [P, C, GW], bf16)
        p2tmp = sb("p2tmp", [P, C, W], bf16)
        p2acc = [sb(f"p2acc{i}", [P, C, W], bf16) for i in range(KH)]
        dist = sb("dist", [P, C, W], f32)
        sg = sb("sg", [P, C, W], f32)
        diff = sb("diff", [P, C, W], f32)
        adiff = sb("adiff", [P, C, W], f32)
        junk = sb("junk", [P, C, W], f32)
        outb = sb("outb", [P, 4], f32)
        warm = sb("warm", [P, 4], f32)
        g2ps = sb("g2ps", [P, C, GW], bf16)
        blks = [ps(f"blk{i}") for i in range(8)]

        dma_sem = ctx.enter_context(nc.semaphore("dma_sem"))
        dve_sem = ctx.enter_context(nc.semaphore("dve_sem"))
        act_sem = ctx.enter_context(nc.semaphore("act_sem"))
        pe_sem = ctx.enter_context(nc.semaphore("pe_sem"))
        w_sem = ctx.enter_context(nc.semaphore("w_sem"))

        block = ctx.enter_context(nc.Block(no_gpsimd_drain=True))

        @block.sync
        def _(sync: "bass.BassEngine"):
            sync.dma_start(out=inp[:], in_=inp_d[:]).then_inc(dma_sem, 16)
            # out DMA (after the DVE chain fully wrote outb)
            sync.wait_ge(dve_sem, 5)
            sync.dma_start(out=out_d[:], in_=outb[:]).then_inc(dma_sem, 16)
            sync.wait_ge(dma_sem, 32)

        @block.scalar
        def _(scalar: "bass.BassEngine"):
            # warm all ACT function tables while the input DMA runs
            scalar.wait_ge(w_sem, 1)
            nc.scalar.copy(warm[:, 1:2], warm[:, 0:1])
            nc.scalar.activation(warm[:, 1:2], warm[:, 0:1], Act.Sigmoid)
            nc.scalar.square(warm[:, 1:2], warm[:, 0:1])
            nc.scalar.sqrt(warm[:, 1:2], warm[:, 0:1])
            scalar.wait_ge(dma_sem, 16)
            nc.scalar.copy(ident[:], inp[:, 2 * C, 0:P]).then_inc(act_sem, 1)  # a=1
            # stage A copies: 8 transpose blocks (b, dr) -> bT, drTp
            k = 0
            for dst, ofs in ((bT, 0), (drTp, 1)):
                for wb in range(C):
                    for hc in range(C):
                        scalar.wait_ge(pe_sem, k + 1)
                        ins = nc.scalar.copy(
                            dst[:, wb, ofs + hc * P : ofs + (hc + 1) * P], blks[k][:]
                        )
                        k += 1
            ins.then_inc(act_sem, 1)  # a=2
            # sigmoid in the idle window (needed only by the DVE tail)
            nc.scalar.activation(sg[:], prd, Act.Sigmoid)
            # square of vertical L1 distance (after DVE doubling: d=3)
            scalar.wait_ge(dve_sem, 3)
            nc.scalar.square(g2T[:], fvB[:, :, PAD1 : PAD1 + H]).then_inc(act_sem, 1)  # a=3
            # stage B copies: 4 transpose blocks g2T -> g2p
            for k in range(4):
                scalar.wait_ge(pe_sem, 9 + k)
                wb, hc = divmod(k, C)
                nc.scalar.copy(g2p[:, hc, KH + wb * P : KH + (wb + 1) * P], blks[k][:])
            # aligned shifted copy: g2ps[j] = g2p[j+1] (even-offset reads for odd u)
            nc.scalar.copy(g2ps[:, :, 0 : GW - 1], g2p[:, :, 1:GW]).then_inc(act_sem, 1)  # a=4
            # sqrt(d2) (after DVE phase 2: d=4)
            scalar.wait_ge(dve_sem, 4)
            nc.scalar.sqrt(dist[:], p2acc[KH - 1][:]).then_inc(act_sem, 1)  # a=5

        @block.tensor
        def _(tensor: "bass.BassEngine"):
            # b_t transposes (DVE d=1) using ident (ACT a=1)
            tensor.wait_ge(act_sem, 1)
            tensor.wait_ge(dve_sem, 1)
            k = 0
            for wb in range(C):
                for hc in range(C):
                    nc.tensor.transpose(
                        blks[k][:], b_t[:, hc, wb * P : (wb + 1) * P], ident[:]
                    ).then_inc(pe_sem, 1)
                    k += 1
            # dr transposes (DVE d=2)
            tensor.wait_ge(dve_sem, 2)
            for wb in range(C):
                for hc in range(C):
                    nc.tensor.transpose(
                        blks[k][:], dr[:, hc, wb * P : (wb + 1) * P], ident[:]
                    ).then_inc(pe_sem, 1)
                    k += 1
            # stage B transposes: g2T (ACT a=3); blks 0-3 reuse gated by a>=2 (covered)
            tensor.wait_ge(act_sem, 3)
            for k in range(4):
                wb, hc = divmod(k, C)
                nc.tensor.transpose(
                    blks[k][:], g2T[:, wb, hc * P : (hc + 1) * P], ident[:]
                ).then_inc(pe_sem, 1)

        @block.vector
        def _(vector: "bass.BassEngine"):
            # data-independent pad memsets first (no waits)
            nc.vector.memset(nbp[:, :, 0:1], 0.0)
            nc.vector.memset(nbp[:, :, W + 1 : W + 2], 0.0)
            nc.vector.memset(drTp[:, :, 0:1], 0.0)
            nc.vector.memset(drTp[:, :, H + 1 : H + 2], 0.0)
            nc.vector.memset(fvA[:, :, 0:PAD1], BIGF)
            nc.vector.memset(fvA[:, :, PAD1 + H : FW], BIGF)
            nc.vector.memset(fvB[:, :, 0:1], BIGF)
            nc.vector.memset(fvB[:, :, FW - 1 : FW], BIGF)
            nc.vector.memset(g2p[:, :, 0:KH], BIG2)
            nc.vector.memset(g2p[:, :, KH + W : GW], BIG2)
            nc.vector.memset(warm[:, 0:1], 1.0).then_inc(w_sem, 1)
            nc.vector.memset(outb[:, 3:4], 0.0)

            vector.wait_ge(dma_sem, 16)
            nc.vector.tensor_scalar(b_t[:], tgt, 0.5, None, Alu.is_gt).then_inc(dve_sem, 1)  # d=1
            nc.vector.tensor_scalar(nbp[:, :, 1 : W + 1], tgt, 0.5, None, Alu.is_le)
            # horizontal dilation of complement
            nc.vector.tensor_tensor(t1[:], nbp[:, :, 0:W], nbp[:, :, 2 : W + 2], Alu.max)
            nc.vector.tensor_tensor(dr[:], t1[:], nbp[:, :, 1 : W + 1], Alu.max).then_inc(dve_sem, 1)  # d=2

            # vertical dilation + boundaries (needs ACT stage A copies: a=2)
            vector.wait_ge(act_sem, 2)
            nc.vector.tensor_tensor(t2[:], drTp[:, :, 0:H], drTp[:, :, 2 : H + 2], Alu.max)
            nc.vector.tensor_tensor(dT[:], t2[:], drTp[:, :, 1 : H + 1], Alu.max)
            nc.vector.tensor_tensor(boundT[:], bT[:], dT[:], Alu.min)
            nc.vector.tensor_scalar(
                fvA[:, :, PAD1 : PAD1 + H], boundT[:], -BIGF, BIGF, Alu.mult, Alu.add
            )
            # vertical L1 distance by log-doubling (window 1+2+4 = 7)
            cur, nxt = fvA, fvB
            for d in (1, 2, 4):
                lo, hi = d, FW - d
                nc.vector.tensor_tensor(
                    tmpd[:, :, lo:hi], cur[:, :, 0 : FW - 2 * d], cur[:, :, 2 * d : FW], Alu.min
                )
                ins = nc.vector.scalar_tensor_tensor(
                    out=nxt[:, :, lo:hi],
                    in0=tmpd[:, :, lo:hi],
                    scalar=float(d),
                    in1=cur[:, :, lo:hi],
                    op0=Alu.add,
                    op1=Alu.min,
                )
                cur, nxt = nxt, cur
            ins.then_inc(dve_sem, 1)  # d=3 (fvB = vertical L1 dist)

            # phase 2 (needs ACT stage B copies + shifted copy: a=4)
            vector.wait_ge(act_sem, 4)
            prev = None
            for u in range(1, KH + 1):
                if u % 2 == 1:
                    in0 = g2ps[:, :, KH - u - 1 : KH - u - 1 + W]
                    in1 = g2ps[:, :, KH + u - 1 : KH + u - 1 + W]
                else:
                    in0 = g2p[:, :, KH - u : KH - u + W]
                    in1 = g2p[:, :, KH + u : KH + u + W]
                nc.vector.tensor_tensor(p2tmp[:], in0, in1, Alu.min)
                base = g2p[:, :, KH : KH + W] if prev is None else prev[:]
                ins = nc.vector.scalar_tensor_tensor(
                    out=p2acc[u - 1][:], in0=p2tmp[:], scalar=float(u * u), in1=base,
                    op0=Alu.add, op1=Alu.min,
                )
                prev = p2acc[u - 1]
            d2 = prev
            ins.then_inc(dve_sem, 1)  # d=4 (d2 ready for ACT sqrt)
            nc.vector.tensor_reduce(
                out=outb[:, 1:3], in_=d2[:], axis=mybir.AxisListType.X, op=Alu.max
            )
            # weighted L1: sum(dist*|diff|) = sum(|dist*diff|) since dist >= 0
            nc.vector.tensor_tensor(diff[:], sg[:], tgt, Alu.subtract)
            vector.wait_ge(act_sem, 5)
            nc.vector.tensor_tensor(junk[:], dist[:], diff[:], Alu.mult)
            nc.vector.tensor_reduce(
                out=outb[:, 0:1], in_=junk[:], axis=mybir.AxisListType.XY, op=Alu.add,
                apply_absolute_value=True,
            ).then_inc(dve_sem, 1)  # d=5 (outb complete)

    return nc


_NC_CACHE = {}


def _get_nc():
    if "nc" not in _NC_CACHE:
        _NC_CACHE["nc"] = _build_nc()
    return _NC_CACHE["nc"]


def _pack_input(tgt_i, prd_i, ident_block):
    # [P, 5*W]: per partition p -> tgt rows p, p+128; pred rows p, p+128; ident row
    return np.concatenate(
        [tgt_i[:P], tgt_i[P:], prd_i[:P], prd_i[P:], ident_block], axis=1
    )


# ---------- exact numpy fallback (pathological images only) ----------

def _reference_image_np(t, p):
    """Exact replica of the jax reference for one image, in numpy fp32."""
    b = (t > 0.5).astype(np.float32)
    if not (b > 0).any():
        return 0.0
    # erode3x3 with +inf border
    v = b.copy()
    v[1:] = np.minimum(v[1:], b[:-1])
    v[:-1] = np.minimum(v[:-1], b[1:])
    er = v.copy()
    er[:, 1:] = np.minimum(er[:, 1:], v[:, :-1])
    er[:, :-1] = np.minimum(er[:, :-1], v[:, 1:])
    bound = b - er
    if bound.sum() == 0:
        bound = b
    feat = bound > 0.5
    BIGV = np.float32(1e6)
    c = np.full(W, BIGV, np.float32)
    d_fwd = np.empty((H, W), np.float32)
    for i in range(H):
        c = np.where(feat[i], np.float32(0.0), c + 1)
        d_fwd[i] = c
    c = np.full(W, BIGV, np.float32)
    d_bwd = np.empty((H, W), np.float32)
    for i in range(H - 1, -1, -1):
        c = np.where(feat[i], np.float32(0.0), c + 1)
        d_bwd[i] = c
    g = np.minimum(d_fwd, d_bwd)
    j = np.arange(W, dtype=np.float32)
    d2 = np.empty((H, W), np.float32)
    for i in range(H):
        d2[i] = np.min(g[i][None, :] ** 2 + (j[:, None] - j[None, :]) ** 2, axis=-1)
    dist = np.sqrt(d2)
    m = dist.max()
    if m > 0:
        dist = dist / (m + np.float32(1e-8))
    sgm = 1.0 / (1.0 + np.exp(-p.astype(np.float64)))
    return float(np.mean(dist * np.abs(sgm - t)))


def _bound_empty(t):
    """True if erosion removes every boundary pixel (reference falls back)."""
    b = (t > 0.5).astype(np.float32)
    v = b.copy()
    v[1:] = np.minimum(v[1:], b[:-1])
    v[:-1] = np.minimum(v[:-1], b[1:])
    er = v.copy()
    er[:, 1:] = np.minimum(er[:, 1:], v[:, :-1])
    er[:, :-1] = np.minimum(er[:, :-1], v[:, 1:])
    return (b - er).sum() == 0


# ---------- public entry point ----------

def kernel(pred_logits: np.ndarray, target: np.ndarray) -> np.ndarray:
    global LAST_RESULTS
    from concourse.bass_utils import run_bass_kernel_spmd

    pred = np.ascontiguousarray(np.asarray(pred_logits, np.float32)[:, 0])
    tgt = np.ascontiguousarray(np.asarray(target, np.float32)[:, 0])
    B = pred.shape[0]
    assert pred.shape == (B, H, W) and tgt.shape == (B, H, W)
    assert B == 8, f"kernel is built for batch 8, got {B}"

    ident_block = np.zeros((P, W), np.float32)
    ident_block[:, :P] = np.eye(P, dtype=np.float32)

    nc = _get_nc()
    in_maps = [{"inp": _pack_input(tgt[i], pred[i], ident_block)} for i in range(B)]
    trace = bool(int(os.environ.get("KERNEL_TRACE", "0")))
    res = run_bass_kernel_spmd(nc, in_maps, core_ids=list(range(B)), trace=trace)
    LAST_RESULTS = res

    total = 0.0
    for i in range(B):
        o = np.asarray(res.results[i]["out"], np.float32)  # [128, 4]
        if not (tgt[i] > 0.5).any():
            continue  # empty mask: reference skips (loss 0)
        m2 = float(o[:, 1:3].max())
        if m2 > float(KH * KH) or _bound_empty(tgt[i]):
            # windowed EDT not provably exact for this image -> exact path
            total += _reference_image_np(tgt[i], pred[i])
            continue
        S = float(o[:, 0].sum(dtype=np.float64))
        m = np.float32(np.sqrt(np.float32(m2)))
        denom = float(m + np.float32(1e-8)) if m > 0 else 1.0
        total += (S / denom) / float(H * W)
    return np.float32(total / max(B, 1))


# revision 23
# speedup vs baseline: 1.2333x; 1.0412x over previous
"""Trainium2 Bass kernel for nn_BoundaryLoss (boundary EDT + weighted L1 loss).

Strategy (pure data parallel, 1 image per NeuronCore, 8 cores):
  Per image on device:
    binary  = target > 0.5
    bound   = binary - erode3x3(binary)          (via complement dilation)
    d2      = windowed exact Euclidean distance transform of bound
              phase 1: vertical L1 distance via log-doubling (window 7)
              phase 2: horizontal parabola min over offsets |u| <= 4
    outputs per partition: sum(sqrt(d2) * |sigmoid(pred)-target|), max(d2)
  Host: final 256-way reduction per image + normalization + batch mean.

Windowed EDT exactness: windowed d2 >= true d2 always, with equality
guaranteed when max(windowed d2) <= K^2 (K = 4): any closer out-of-window
feature would have |di|,|dj| < K and hence be in-window.  The device
returns max(d2); the host verifies the bound and falls back to an exact
numpy path for any image that fails it (never on dense masks).

Raw bass (no Tile): the pipeline is linear across 4 engines (DVE chain,
ACT helper ops, PE transposes, SP DMA), so stage-boundary semaphores are
enough, every instruction carries <= 2 sync waits (ISA limit), and there
is no Tile kernel-tail barrier overhead.

All inputs ship as ONE DRAM tensor (target rows 0:256, pred rows 256:512,
an f32 identity block rows 512:640) so a single input DMA feeds the core.
"""

import os
from contextlib import ExitStack

import numpy as np

H = 256
W = 256
P = 128
C = 2  # partition chunks per image (H = C * P)
KH = 4  # phase-2 horizontal window (exactness proof bound: m2 <= KH*KH)
BIGF = 16384.0  # phase-1 sentinel (bf16-exact; BIGF + small stays BIGF in bf16)
BIG2 = 3.0e8  # phase-2 border pad, > BIGF^2
PAD1 = 8  # phase-1 doubling pads (window 1+2+4; 8 keeps slices 4B-aligned)
FW = H + 2 * PAD1
GW = W + 2 * KH

LAST_RESULTS = None  # BassKernelResults of the most recent device run


def _build_nc():
    import concourse.bass as bass
    import concourse.mybir as mybir

    bf16 = mybir.dt.bfloat16
    f32 = mybir.dt.float32
    Alu = mybir.AluOpType
    Act = mybir.ActivationFunctionType

    nc = bass.Bass(detect_race_conditions=False)
    inp_d = nc.dram_tensor("inp", [P, 5 * W], f32, kind="ExternalInput")
    out_d = nc.dram_tensor("out", [P, 4], f32, kind="ExternalOutput")

    ctx = ExitStack()
    sb = lambda name, shape, dt: ctx.enter_context(nc.sbuf_tensor(name, shape, dt))
    ps = lambda name: ctx.enter_context(nc.psum_tensor(name, [P, P], bf16))

    with ctx:
        inp = sb("inp_t", [P, 5, W], f32)
        tgt = inp[:, 0:C, :]
        prd = inp[:, C : 2 * C, :]
        ident = sb("ident", [P, P], bf16)
        nbp = sb("nbp", [P, C, W + 2], bf16)
        b_t = sb("b_t", [P, C, W], bf16)
        t1 = sb("t1", [P, C, W], bf16)
        dr = sb("dr", [P, C, W], bf16)
        bT = sb("bT", [P, C, H], bf16)
        drTp = sb("drTp", [P, C, H + 2], bf16)
        t2 = sb("t2", [P, C, H], bf16)
        dT = sb("dT", [P, C, H], bf16)
        boundT = sb("boundT", [P, C, H], bf16)
        fvA = sb("fvA", [P, C, FW], bf16)
        fvB = sb("fvB", [P, C, FW], bf16)
        tmpd = sb("tmpd", [P, C, FW], bf16)
        g2T = sb("g2T", [P, C, H], bf16)
        g2p = sb("g2p", [P, C, GW], bf16)
        p2tmp = sb("p2tmp", [P, C, W], bf16)
        p2acc = [sb(f"p2acc{i}", [P, C, W], bf16) for i in range(KH)]
        dist = sb("dist", [P, C, W], f32)
        sg = sb("sg", [P, C, W], f32)
        diff = sb("diff", [P, C, W], f32)
        adiff = sb("adiff", [P, C, W], f32)
        junk = sb("junk", [P, C, W], f32)
        outb = sb("outb", [P, 4], f32)
        warm = sb("warm", [P, 4], f32)
        g2ps = sb("g2ps", [P, C, GW], bf16)
        blks = [ps(f"blk{i}") for i in range(8)]

        dma_sem = ctx.enter_context(nc.semaphore("dma_sem"))
        dve_sem = ctx.enter_context(nc.semaphore("dve_sem"))
        act_sem = ctx.enter_context(nc.semaphore("act_sem"))
        pe_sem = ctx.enter_context(nc.semaphore("pe_sem"))
        w_sem = ctx.enter_context(nc.semaphore("w_sem"))
        dma2_sem = ctx.enter_context(nc.semaphore("dma2_sem"))

        block = ctx.enter_context(nc.Block(no_gpsimd_drain=True))

        @block.sync
        def _(sync: "bass.BassEngine"):
            # target half of the input (pred+ident half goes via the ACT HWDGE)
            sync.dma_start(out=inp[:, 0:C, :], in_=inp_d[:, 0 : C * W]).then_inc(dma_sem, 16)
            # out DMA (after the DVE chain fully wrote outb)
            sync.wait_ge(dve_sem, 5)
            sync.dma_start(out=out_d[:], in_=outb[:]).then_inc(dma_sem, 16)
            sync.wait_ge(dma_sem, 32)

        @block.scalar
        def _(scalar: "bass.BassEngine"):
            # pred + identity half of the input on the ACT HWDGE queue
            nc.scalar.dma_start(out=inp[:, C:5, :], in_=inp_d[:, C * W :]).then_inc(dma2_sem, 16)
            # warm the ACT function tables while the DMAs run
            scalar.wait_ge(w_sem, 1)
            nc.scalar.sqrt(warm[:, 1:2], warm[:, 0:1])
            nc.scalar.activation(warm[:, 1:2], warm[:, 0:1], Act.Sigmoid)
            nc.scalar.copy(warm[:, 1:2], warm[:, 0:1])
            scalar.wait_ge(dma2_sem, 16)
            nc.scalar.copy(ident[:], inp[:, 2 * C, 0:P]).then_inc(act_sem, 1)  # a=1
            # bT copies: 4 transpose blocks (DVE copies the dr blocks itself)
            k = 0
            for wb in range(C):
                for hc in range(C):
                    scalar.wait_ge(pe_sem, k + 1)
                    ins = nc.scalar.copy(bT[:, wb, hc * P : (hc + 1) * P], blks[k][:])
                    k += 1
            ins.then_inc(act_sem, 1)  # a=2
            # sigmoid in the idle window (needed only by the DVE tail)
            nc.scalar.activation(sg[:], prd, Act.Sigmoid)
            # stage B copies: 4 transpose blocks g2T -> g2p
            for k in range(4):
                scalar.wait_ge(pe_sem, 9 + k)
                wb, hc = divmod(k, C)
                nc.scalar.copy(g2p[:, hc, KH + wb * P : KH + (wb + 1) * P], blks[k][:])
            # aligned shifted copy: g2ps[j] = g2p[j+1] (even-offset reads for odd u)
            nc.scalar.copy(g2ps[:, :, 0 : GW - 1], g2p[:, :, 1:GW])
            # re-warm the sqrt table so the real sqrt issues immediately
            nc.scalar.sqrt(warm[:, 2:3], warm[:, 0:1]).then_inc(act_sem, 1)  # a=3
            # sqrt(d2) (after DVE phase 2: d=4)
            scalar.wait_ge(dve_sem, 4)
            nc.scalar.sqrt(dist[:], p2acc[KH - 1][:]).then_inc(act_sem, 1)  # a=4

        @block.tensor
        def _(tensor: "bass.BassEngine"):
            # b_t transposes (DVE d=1) using ident (ACT a=1)
            tensor.wait_ge(act_sem, 1)
            tensor.wait_ge(dve_sem, 1)
            k = 0
            for wb in range(C):
                for hc in range(C):
                    nc.tensor.transpose(
                        blks[k][:], b_t[:, hc, wb * P : (wb + 1) * P], ident[:]
                    ).then_inc(pe_sem, 1)
                    k += 1
            # dr transposes (DVE d=2)
            tensor.wait_ge(dve_sem, 2)
            for wb in range(C):
                for hc in range(C):
                    nc.tensor.transpose(
                        blks[k][:], dr[:, hc, wb * P : (wb + 1) * P], ident[:]
                    ).then_inc(pe_sem, 1)
                    k += 1
            # stage B transposes: g2T (DVE d=3); blks 0-3 readers done at a>=2
            tensor.wait_ge(dve_sem, 3)
            tensor.wait_ge(act_sem, 2)
            for k in range(4):
                wb, hc = divmod(k, C)
                nc.tensor.transpose(
                    blks[k][:], g2T[:, wb, hc * P : (hc + 1) * P], ident[:]
                ).then_inc(pe_sem, 1)

        @block.vector
        def _(vector: "bass.BassEngine"):
            # data-independent pad memsets first (no waits)
            nc.vector.memset(nbp[:, :, 0:1], 0.0)
            nc.vector.memset(nbp[:, :, W + 1 : W + 2], 0.0)
            nc.vector.memset(drTp[:, :, 0:1], 0.0)
            nc.vector.memset(drTp[:, :, H + 1 : H + 2], 0.0)
            nc.vector.memset(fvA[:, :, 0:PAD1], BIGF)
            nc.vector.memset(fvA[:, :, PAD1 + H : FW], BIGF)
            nc.vector.memset(fvB[:, :, 0:1], BIGF)
            nc.vector.memset(fvB[:, :, FW - 1 : FW], BIGF)
            nc.vector.memset(g2p[:, :, 0:KH], BIG2)
            nc.vector.memset(g2p[:, :, KH + W : GW], BIG2)
            nc.vector.memset(outb[:, 3:4], 0.0)
            nc.vector.memset(warm[:, 0:1], 1.0).then_inc(w_sem, 1)

            vector.wait_ge(dma_sem, 16)
            nc.vector.tensor_scalar(b_t[:], tgt, 0.5, None, Alu.is_gt).then_inc(dve_sem, 1)  # d=1
            nc.vector.tensor_scalar(nbp[:, :, 1 : W + 1], tgt, 0.5, None, Alu.is_le)
            # horizontal dilation of complement
            nc.vector.tensor_tensor(t1[:], nbp[:, :, 0:W], nbp[:, :, 2 : W + 2], Alu.max)
            nc.vector.tensor_tensor(dr[:], t1[:], nbp[:, :, 1 : W + 1], Alu.max).then_inc(dve_sem, 1)  # d=2

            # copy the dr transpose blocks from PSUM ourselves (ACT does bT)
            for k in range(4):
                vector.wait_ge(pe_sem, 5 + k)
                wb, hc = divmod(k, C)
                nc.vector.tensor_copy(drTp[:, wb, 1 + hc * P : 1 + (hc + 1) * P], blks[4 + k][:])
            # vertical dilation + boundaries
            nc.vector.tensor_tensor(t2[:], drTp[:, :, 0:H], drTp[:, :, 2 : H + 2], Alu.max)
            nc.vector.tensor_tensor(dT[:], t2[:], drTp[:, :, 1 : H + 1], Alu.max)
            vector.wait_ge(act_sem, 2)
            nc.vector.tensor_tensor(boundT[:], bT[:], dT[:], Alu.min)
            nc.vector.tensor_scalar(
                fvA[:, :, PAD1 : PAD1 + H], boundT[:], -BIGF, BIGF, Alu.mult, Alu.add
            )
            # vertical L1 distance by log-doubling (window 1+2+4 = 7)
            cur, nxt = fvA, fvB
            for d in (1, 2, 4):
                lo, hi = d, FW - d
                nc.vector.tensor_tensor(
                    tmpd[:, :, lo:hi], cur[:, :, 0 : FW - 2 * d], cur[:, :, 2 * d : FW], Alu.min
                )
                nc.vector.scalar_tensor_tensor(
                    out=nxt[:, :, lo:hi],
                    in0=tmpd[:, :, lo:hi],
                    scalar=float(d),
                    in1=cur[:, :, lo:hi],
                    op0=Alu.add,
                    op1=Alu.min,
                )
                cur, nxt = nxt, cur
            # square the vertical L1 distance (cur = fvB interior)
            nc.vector.tensor_tensor(
                g2T[:], cur[:, :, PAD1 : PAD1 + H], cur[:, :, PAD1 : PAD1 + H], Alu.mult
            ).then_inc(dve_sem, 1)  # d=3

            # phase 2 (needs ACT stage B copies + shifted copy: a=3)
            vector.wait_ge(act_sem, 3)
            prev = None
            for u in range(1, KH + 1):
                if u % 2 == 1:
                    in0 = g2ps[:, :, KH - u - 1 : KH - u - 1 + W]
                    in1 = g2ps[:, :, KH + u - 1 : KH + u - 1 + W]
                else:
                    in0 = g2p[:, :, KH - u : KH - u + W]
                    in1 = g2p[:, :, KH + u : KH + u + W]
                nc.vector.tensor_tensor(p2tmp[:], in0, in1, Alu.min)
                base = g2p[:, :, KH : KH + W] if prev is None else prev[:]
                ins = nc.vector.scalar_tensor_tensor(
                    out=p2acc[u - 1][:], in0=p2tmp[:], scalar=float(u * u), in1=base,
                    op0=Alu.add, op1=Alu.min,
                )
                prev = p2acc[u - 1]
            d2 = prev
            ins.then_inc(dve_sem, 1)  # d=4 (d2 ready for ACT sqrt)
            nc.vector.tensor_reduce(
                out=outb[:, 1:3], in_=d2[:], axis=mybir.AxisListType.X, op=Alu.max
            )
            # weighted L1: sum(dist*|diff|) = sum(|dist*diff|) since dist >= 0
            nc.vector.tensor_tensor(diff[:], sg[:], tgt, Alu.subtract)
            vector.wait_ge(act_sem, 4)
            nc.vector.tensor_tensor(junk[:], dist[:], diff[:], Alu.mult)
            nc.vector.tensor_reduce(
                out=outb[:, 0:1], in_=junk[:], axis=mybir.AxisListType.XY, op=Alu.add,
                apply_absolute_value=True,
            ).then_inc(dve_sem, 1)  # d=5 (outb complete)

    return nc


_NC_CACHE = {}


def _get_nc():
    if "nc" not in _NC_CACHE:
        _NC_CACHE["nc"] = _build_nc()
    return _NC_CACHE["nc"]


def _pack_input(tgt_i, prd_i, ident_block):
    # [P, 5*W]: per partition p -> tgt rows p, p+128; pred rows p, p+128; ident row
    return np.concatenate(
        [tgt_i[:P], tgt_i[P:], prd_i[:P], prd_i[P:], ident_block], axis=1
    )


# ---------- exact numpy fallback (pathological images only) ----------

def _reference_image_np(t, p):
    """Exact replica of the jax reference for one image, in numpy fp32."""
    b = (t > 0.5).astype(np.float32)
    if not (b > 0).any():
        return 0.0
    # erode3x3 with +inf border
    v = b.copy()
    v[1:] = np.minimum(v[1:], b[:-1])
    v[:-1] = np.minimum(v[:-1], b[1:])
    er = v.copy()
    er[:, 1:] = np.minimum(er[:, 1:], v[:, :-1])
    er[:, :-1] = np.minimum(er[:, :-1], v[:, 1:])
    bound = b - er
    if bound.sum() == 0:
        bound = b
    feat = bound > 0.5
    BIGV = np.float32(1e6)
    c = np.full(W, BIGV, np.float32)
    d_fwd = np.empty((H, W), np.float32)
    for i in range(H):
        c = np.where(feat[i], np.float32(0.0), c + 1)
        d_fwd[i] = c
    c = np.full(W, BIGV, np.float32)
    d_bwd = np.empty((H, W), np.float32)
    for i in range(H - 1, -1, -1):
        c = np.where(feat[i], np.float32(0.0), c + 1)
        d_bwd[i] = c
    g = np.minimum(d_fwd, d_bwd)
    j = np.arange(W, dtype=np.float32)
    d2 = np.empty((H, W), np.float32)
    for i in range(H):
        d2[i] = np.min(g[i][None, :] ** 2 + (j[:, None] - j[None, :]) ** 2, axis=-1)
    dist = np.sqrt(d2)
    m = dist.max()
    if m > 0:
        dist = dist / (m + np.float32(1e-8))
    sgm = 1.0 / (1.0 + np.exp(-p.astype(np.float64)))
    return float(np.mean(dist * np.abs(sgm - t)))


def _bound_empty(t):
    """True if erosion removes every boundary pixel (reference falls back)."""
    b = (t > 0.5).astype(np.float32)
    v = b.copy()
    v[1:] = np.minimum(v[1:], b[:-1])
    v[:-1] = np.minimum(v[:-1], b[1:])
    er = v.copy()
    er[:, 1:] = np.minimum(er[:, 1:], v[:, :-1])
    er[:, :-1] = np.minimum(er[:, :-1], v[:, 1:])
    return (b - er).sum() == 0


# ---------- public entry point ----------

def kernel(pred_logits: np.ndarray, target: np.ndarray) -> np.ndarray:
    global LAST_RESULTS
    from concourse.bass_utils import run_bass_kernel_spmd

    pred = np.ascontiguousarray(np.asarray(pred_logits, np.float32)[:, 0])
    tgt = np.ascontiguousarray(np.asarray(target, np.float32)[:, 0])
    B = pred.shape[0]
    assert pred.shape == (B, H, W) and tgt.shape == (B, H, W)
    assert B == 8, f"kernel is built for batch 8, got {B}"

    ident_block = np.zeros((P, W), np.float32)
    ident_block[:, :P] = np.eye(P, dtype=np.float32)

    nc = _get_nc()
    in_maps = [{"inp": _pack_input(tgt[i], pred[i], ident_block)} for i in range(B)]
    trace = bool(int(os.environ.get("KERNEL_TRACE", "0")))
    res = run_bass_kernel_spmd(nc, in_maps, core_ids=list(range(B)), trace=trace)
    LAST_RESULTS = res

    total = 0.0
    for i in range(B):
        o = np.asarray(res.results[i]["out"], np.float32)  # [128, 4]
        if not (tgt[i] > 0.5).any():
            continue  # empty mask: reference skips (loss 0)
        m2 = float(o[:, 1:3].max())
        if m2 > float(KH * KH) or _bound_empty(tgt[i]):
            # windowed EDT not provably exact for this image -> exact path
            total += _reference_image_np(tgt[i], pred[i])
            continue
        S = float(o[:, 0].sum(dtype=np.float64))
        m = np.float32(np.sqrt(np.float32(m2)))
        denom = float(m + np.float32(1e-8)) if m > 0 else 1.0
        total += (S / denom) / float(H * W)
    return np.float32(total / max(B, 1))


# revision 24
# speedup vs baseline: 1.3546x; 1.0984x over previous
"""Trainium2 Bass kernel for nn_BoundaryLoss (boundary EDT + weighted L1 loss).

Strategy (pure data parallel, 1 image per NeuronCore, 8 cores):
  Per image on device:
    binary  = target > 0.5
    bound   = binary - erode3x3(binary)          (via complement dilation)
    d2      = windowed exact Euclidean distance transform of bound
              phase 1: vertical L1 distance via log-doubling (window 7)
              phase 2: horizontal parabola min over offsets |u| <= 4
    outputs per partition: sum(sqrt(d2) * |sigmoid(pred)-target|), max(d2)
  Host: final 256-way reduction per image + normalization + batch mean.

Windowed EDT exactness: windowed d2 >= true d2 always, with equality
guaranteed when max(windowed d2) <= K^2 (K = 4): any closer out-of-window
feature would have |di|,|dj| < K and hence be in-window.  The device
returns max(d2); the host verifies the bound and falls back to an exact
numpy path for any image that fails it (never on dense masks).

Raw bass (no Tile): the pipeline is linear across 4 engines (DVE chain,
ACT helper ops, PE transposes, SP DMA), so stage-boundary semaphores are
enough, every instruction carries <= 2 sync waits (ISA limit), and there
is no Tile kernel-tail barrier overhead.

All inputs ship as ONE DRAM tensor (target rows 0:256, pred rows 256:512,
an f32 identity block rows 512:640) so a single input DMA feeds the core.
"""

import os
from contextlib import ExitStack

import numpy as np

H = 256
W = 256
P = 128
C = 2  # partition chunks per image (H = C * P)
KH = 4  # phase-2 horizontal window (exactness proof bound: m2 <= KH*KH)
BIGF = 16384.0  # phase-1 sentinel (bf16-exact; BIGF + small stays BIGF in bf16)
BIG2 = 3.0e8  # phase-2 border pad, > BIGF^2
PAD1 = 8  # phase-1 doubling pads (window 1+2+4; 8 keeps slices 4B-aligned)
FW = H + 2 * PAD1
GW = W + 2 * KH

LAST_RESULTS = None  # BassKernelResults of the most recent device run


def _build_nc():
    import concourse.bass as bass
    import concourse.mybir as mybir

    bf16 = mybir.dt.bfloat16
    f32 = mybir.dt.float32
    Alu = mybir.AluOpType
    Act = mybir.ActivationFunctionType

    nc = bass.Bass(detect_race_conditions=False)
    inp_d = nc.dram_tensor("inp", [P, 5 * W], f32, kind="ExternalInput")
    out_d = nc.dram_tensor("out", [P, 4], f32, kind="ExternalOutput")

    ctx = ExitStack()
    sb = lambda name, shape, dt: ctx.enter_context(nc.sbuf_tensor(name, shape, dt))
    ps = lambda name: ctx.enter_context(nc.psum_tensor(name, [P, P], bf16))

    with ctx:
        inp = sb("inp_t", [P, 5, W], f32)
        tgt = inp[:, 0:C, :]
        prd = inp[:, C + 1 : 5, :]
        ident = sb("ident", [P, P], bf16)
        nbp = sb("nbp", [P, C, W + 2], bf16)
        b_t = sb("b_t", [P, C, W], bf16)
        t1 = sb("t1", [P, C, W], bf16)
        dr = sb("dr", [P, C, W], bf16)
        bT = sb("bT", [P, C, H], bf16)
        drTp = sb("drTp", [P, C, H + 2], bf16)
        t2 = sb("t2", [P, C, H], bf16)
        dT = sb("dT", [P, C, H], bf16)
        boundT = sb("boundT", [P, C, H], bf16)
        fvA = sb("fvA", [P, C, FW], bf16)
        fvB = sb("fvB", [P, C, FW], bf16)
        tmpd = sb("tmpd", [P, C, FW], bf16)
        g2T = sb("g2T", [P, C, H], bf16)
        g2p = sb("g2p", [P, C, GW], bf16)
        p2tmp = sb("p2tmp", [P, C, W], bf16)
        p2acc = [sb(f"p2acc{i}", [P, C, W], bf16) for i in range(KH)]
        dist = sb("dist", [P, C, W], f32)
        sg = sb("sg", [P, C, W], f32)
        diff = sb("diff", [P, C, W], f32)
        adiff = sb("adiff", [P, C, W], f32)
        junk = sb("junk", [P, C, W], f32)
        outb = sb("outb", [P, 4], f32)
        warm = sb("warm", [P, 4], f32)
        g2ps = sb("g2ps", [P, C, GW], bf16)
        blks = [ps(f"blk{i}") for i in range(8)]

        dma_sem = ctx.enter_context(nc.semaphore("dma_sem"))
        dve_sem = ctx.enter_context(nc.semaphore("dve_sem"))
        act_sem = ctx.enter_context(nc.semaphore("act_sem"))
        pe_sem = ctx.enter_context(nc.semaphore("pe_sem"))
        w_sem = ctx.enter_context(nc.semaphore("w_sem"))
        dma2_sem = ctx.enter_context(nc.semaphore("dma2_sem"))

        block = ctx.enter_context(nc.Block(no_gpsimd_drain=True))

        @block.sync
        def _(sync: "bass.BassEngine"):
            # target half of the input (pred+ident half goes via the ACT HWDGE)
            sync.dma_start(out=inp[:, 0 : C + 1, :], in_=inp_d[:, 0 : (C + 1) * W]).then_inc(dma_sem, 16)
            # out DMA (after the DVE chain fully wrote outb)
            sync.wait_ge(dve_sem, 5)
            sync.dma_start(out=out_d[:], in_=outb[:]).then_inc(dma_sem, 16)
            sync.wait_ge(dma_sem, 32)

        @block.scalar
        def _(scalar: "bass.BassEngine"):
            # pred + identity half of the input on the ACT HWDGE queue
            nc.scalar.dma_start(out=inp[:, C + 1 : 5, :], in_=inp_d[:, (C + 1) * W :]).then_inc(dma2_sem, 16)
            # warm the ACT function tables while the DMAs run
            scalar.wait_ge(w_sem, 1)
            nc.scalar.sqrt(warm[:, 1:2], warm[:, 0:1])
            nc.scalar.activation(warm[:, 1:2], warm[:, 0:1], Act.Sigmoid)
            nc.scalar.copy(warm[:, 1:2], warm[:, 0:1])
            scalar.wait_ge(dma_sem, 16)
            nc.scalar.copy(ident[:], inp[:, C, 0:P]).then_inc(act_sem, 1)  # a=1
            # bT copies: 4 transpose blocks (DVE copies the dr blocks itself)
            k = 0
            for wb in range(C):
                for hc in range(C):
                    scalar.wait_ge(pe_sem, k + 1)
                    ins = nc.scalar.copy(bT[:, wb, hc * P : (hc + 1) * P], blks[k][:])
                    k += 1
            ins.then_inc(act_sem, 1)  # a=2
            # sigmoid in the idle window (needed only by the DVE tail)
            scalar.wait_ge(dma2_sem, 16)
            nc.scalar.activation(sg[:], prd, Act.Sigmoid)
            # re-warm the sqrt table so the real sqrt issues immediately
            nc.scalar.sqrt(warm[:, 2:3], warm[:, 0:1])
            # sqrt(d2) (after DVE phase 2: d=4)
            scalar.wait_ge(dve_sem, 4)
            nc.scalar.sqrt(dist[:], p2acc[KH - 1][:]).then_inc(act_sem, 1)  # a=3

        @block.tensor
        def _(tensor: "bass.BassEngine"):
            # b_t transposes (DVE d=1) using ident (ACT a=1)
            tensor.wait_ge(act_sem, 1)
            tensor.wait_ge(dve_sem, 1)
            k = 0
            for wb in range(C):
                for hc in range(C):
                    nc.tensor.transpose(
                        blks[k][:], b_t[:, hc, wb * P : (wb + 1) * P], ident[:]
                    ).then_inc(pe_sem, 1)
                    k += 1
            # dr transposes (DVE d=2)
            tensor.wait_ge(dve_sem, 2)
            for wb in range(C):
                for hc in range(C):
                    nc.tensor.transpose(
                        blks[k][:], dr[:, hc, wb * P : (wb + 1) * P], ident[:]
                    ).then_inc(pe_sem, 1)
                    k += 1
            # stage B transposes: g2T (DVE d=3); blks 0-3 readers done at a>=2
            tensor.wait_ge(dve_sem, 3)
            tensor.wait_ge(act_sem, 2)
            for k in range(4):
                wb, hc = divmod(k, C)
                nc.tensor.transpose(
                    blks[k][:], g2T[:, wb, hc * P : (hc + 1) * P], ident[:]
                ).then_inc(pe_sem, 1)

        @block.vector
        def _(vector: "bass.BassEngine"):
            # data-independent pad memsets first (no waits)
            nc.vector.memset(nbp[:, :, 0:1], 0.0)
            nc.vector.memset(nbp[:, :, W + 1 : W + 2], 0.0)
            nc.vector.memset(drTp[:, :, 0:1], 0.0)
            nc.vector.memset(drTp[:, :, H + 1 : H + 2], 0.0)
            nc.vector.memset(fvA[:, :, 0:PAD1], BIGF)
            nc.vector.memset(fvA[:, :, PAD1 + H : FW], BIGF)
            nc.vector.memset(fvB[:, :, 0:1], BIGF)
            nc.vector.memset(fvB[:, :, FW - 1 : FW], BIGF)
            nc.vector.memset(g2p[:, :, 0:KH], BIG2)
            nc.vector.memset(g2p[:, :, KH + W : GW], BIG2)
            nc.vector.memset(outb[:, 3:4], 0.0)
            nc.vector.memset(warm[:, 0:1], 1.0).then_inc(w_sem, 1)

            vector.wait_ge(dma_sem, 16)
            nc.vector.tensor_scalar(b_t[:], tgt, 0.5, None, Alu.is_gt).then_inc(dve_sem, 1)  # d=1
            nc.vector.tensor_scalar(nbp[:, :, 1 : W + 1], tgt, 0.5, None, Alu.is_le)
            # horizontal dilation of complement
            nc.vector.tensor_tensor(t1[:], nbp[:, :, 0:W], nbp[:, :, 2 : W + 2], Alu.max)
            nc.vector.tensor_tensor(dr[:], t1[:], nbp[:, :, 1 : W + 1], Alu.max).then_inc(dve_sem, 1)  # d=2

            # copy the dr transpose blocks from PSUM ourselves (ACT does bT)
            for k in range(4):
                vector.wait_ge(pe_sem, 5 + k)
                wb, hc = divmod(k, C)
                nc.vector.tensor_copy(drTp[:, wb, 1 + hc * P : 1 + (hc + 1) * P], blks[4 + k][:])
            # vertical dilation + boundaries
            nc.vector.tensor_tensor(t2[:], drTp[:, :, 0:H], drTp[:, :, 2 : H + 2], Alu.max)
            nc.vector.tensor_tensor(dT[:], t2[:], drTp[:, :, 1 : H + 1], Alu.max)
            vector.wait_ge(act_sem, 2)
            nc.vector.tensor_tensor(boundT[:], bT[:], dT[:], Alu.min)
            nc.vector.tensor_scalar(
                fvA[:, :, PAD1 : PAD1 + H], boundT[:], -BIGF, BIGF, Alu.mult, Alu.add
            )
            # vertical L1 distance by log-doubling (window 1+2+4 = 7)
            cur, nxt = fvA, fvB
            for d in (1, 2, 4):
                lo, hi = d, FW - d
                nc.vector.tensor_tensor(
                    tmpd[:, :, lo:hi], cur[:, :, 0 : FW - 2 * d], cur[:, :, 2 * d : FW], Alu.min
                )
                nc.vector.scalar_tensor_tensor(
                    out=nxt[:, :, lo:hi],
                    in0=tmpd[:, :, lo:hi],
                    scalar=float(d),
                    in1=cur[:, :, lo:hi],
                    op0=Alu.add,
                    op1=Alu.min,
                )
                cur, nxt = nxt, cur
            # square the vertical L1 distance (cur = fvB interior)
            nc.vector.tensor_tensor(
                g2T[:], cur[:, :, PAD1 : PAD1 + H], cur[:, :, PAD1 : PAD1 + H], Alu.mult
            ).then_inc(dve_sem, 1)  # d=3

            # stage B copies: 4 transpose blocks g2T -> g2p, then shifted copy
            for k in range(4):
                vector.wait_ge(pe_sem, 9 + k)
                wb, hc = divmod(k, C)
                nc.vector.tensor_copy(g2p[:, hc, KH + wb * P : KH + (wb + 1) * P], blks[k][:])
            # aligned shifted copy: g2ps[j] = g2p[j+1] (even-offset reads for odd u)
            nc.vector.tensor_copy(g2ps[:, :, 0 : GW - 1], g2p[:, :, 1:GW])
            # phase 2
            prev = None
            for u in range(1, KH + 1):
                if u % 2 == 1:
                    in0 = g2ps[:, :, KH - u - 1 : KH - u - 1 + W]
                    in1 = g2ps[:, :, KH + u - 1 : KH + u - 1 + W]
                else:
                    in0 = g2p[:, :, KH - u : KH - u + W]
                    in1 = g2p[:, :, KH + u : KH + u + W]
                nc.vector.tensor_tensor(p2tmp[:], in0, in1, Alu.min)
                base = g2p[:, :, KH : KH + W] if prev is None else prev[:]
                ins = nc.vector.scalar_tensor_tensor(
                    out=p2acc[u - 1][:], in0=p2tmp[:], scalar=float(u * u), in1=base,
                    op0=Alu.add, op1=Alu.min,
                )
                prev = p2acc[u - 1]
            d2 = prev
            ins.then_inc(dve_sem, 1)  # d=4 (d2 ready for ACT sqrt)
            nc.vector.tensor_reduce(
                out=outb[:, 1:3], in_=d2[:], axis=mybir.AxisListType.X, op=Alu.max
            )
            # weighted L1: sum(dist*|diff|) = sum(|dist*diff|) since dist >= 0
            nc.vector.tensor_tensor(diff[:], sg[:], tgt, Alu.subtract)
            vector.wait_ge(act_sem, 3)
            nc.vector.tensor_tensor(junk[:], dist[:], diff[:], Alu.mult)
            nc.vector.tensor_reduce(
                out=outb[:, 0:1], in_=junk[:], axis=mybir.AxisListType.XY, op=Alu.add,
                apply_absolute_value=True,
            ).then_inc(dve_sem, 1)  # d=5 (outb complete)

    return nc


_NC_CACHE = {}


def _get_nc():
    if "nc" not in _NC_CACHE:
        _NC_CACHE["nc"] = _build_nc()
    return _NC_CACHE["nc"]


def _pack_input(tgt_i, prd_i, ident_block):
    # [P, 5*W]: per partition p -> tgt rows p, p+128; ident row; pred rows p, p+128
    return np.concatenate(
        [tgt_i[:P], tgt_i[P:], ident_block, prd_i[:P], prd_i[P:]], axis=1
    )


# ---------- exact numpy fallback (pathological images only) ----------

def _reference_image_np(t, p):
    """Exact replica of the jax reference for one image, in numpy fp32."""
    b = (t > 0.5).astype(np.float32)
    if not (b > 0).any():
        return 0.0
    # erode3x3 with +inf border
    v = b.copy()
    v[1:] = np.minimum(v[1:], b[:-1])
    v[:-1] = np.minimum(v[:-1], b[1:])
    er = v.copy()
    er[:, 1:] = np.minimum(er[:, 1:], v[:, :-1])
    er[:, :-1] = np.minimum(er[:, :-1], v[:, 1:])
    bound = b - er
    if bound.sum() == 0:
        bound = b
    feat = bound > 0.5
    BIGV = np.float32(1e6)
    c = np.full(W, BIGV, np.float32)
    d_fwd = np.empty((H, W), np.float32)
    for i in range(H):
        c = np.where(feat[i], np.float32(0.0), c + 1)
        d_fwd[i] = c
    c = np.full(W, BIGV, np.float32)
    d_bwd = np.empty((H, W), np.float32)
    for i in range(H - 1, -1, -1):
        c = np.where(feat[i], np.float32(0.0), c + 1)
        d_bwd[i] = c
    g = np.minimum(d_fwd, d_bwd)
    j = np.arange(W, dtype=np.float32)
    d2 = np.empty((H, W), np.float32)
    for i in range(H):
        d2[i] = np.min(g[i][None, :] ** 2 + (j[:, None] - j[None, :]) ** 2, axis=-1)
    dist = np.sqrt(d2)
    m = dist.max()
    if m > 0:
        dist = dist / (m + np.float32(1e-8))
    sgm = 1.0 / (1.0 + np.exp(-p.astype(np.float64)))
    return float(np.mean(dist * np.abs(sgm - t)))


def _bound_empty(t):
    """True if erosion removes every boundary pixel (reference falls back)."""
    b = (t > 0.5).astype(np.float32)
    v = b.copy()
    v[1:] = np.minimum(v[1:], b[:-1])
    v[:-1] = np.minimum(v[:-1], b[1:])
    er = v.copy()
    er[:, 1:] = np.minimum(er[:, 1:], v[:, :-1])
    er[:, :-1] = np.minimum(er[:, :-1], v[:, 1:])
    return (b - er).sum() == 0


# ---------- public entry point ----------

def kernel(pred_logits: np.ndarray, target: np.ndarray) -> np.ndarray:
    global LAST_RESULTS
    from concourse.bass_utils import run_bass_kernel_spmd

    pred = np.ascontiguousarray(np.asarray(pred_logits, np.float32)[:, 0])
    tgt = np.ascontiguousarray(np.asarray(target, np.float32)[:, 0])
    B = pred.shape[0]
    assert pred.shape == (B, H, W) and tgt.shape == (B, H, W)
    assert B == 8, f"kernel is built for batch 8, got {B}"

    ident_block = np.zeros((P, W), np.float32)
    ident_block[:, :P] = np.eye(P, dtype=np.float32)

    nc = _get_nc()
    in_maps = [{"inp": _pack_input(tgt[i], pred[i], ident_block)} for i in range(B)]
    trace = bool(int(os.environ.get("KERNEL_TRACE", "0")))
    res = run_bass_kernel_spmd(nc, in_maps, core_ids=list(range(B)), trace=trace)
    LAST_RESULTS = res

    total = 0.0
    for i in range(B):
        o = np.asarray(res.results[i]["out"], np.float32)  # [128, 4]
        if not (tgt[i] > 0.5).any():
            continue  # empty mask: reference skips (loss 0)
        m2 = float(o[:, 1:3].max())
        if m2 > float(KH * KH) or _bound_empty(tgt[i]):
            # windowed EDT not provably exact for this image -> exact path
            total += _reference_image_np(tgt[i], pred[i])
            continue
        S = float(o[:, 0].sum(dtype=np.float64))
        m = np.float32(np.sqrt(np.float32(m2)))
        denom = float(m + np.float32(1e-8)) if m > 0 else 1.0
        total += (S / denom) / float(H * W)
    return np.float32(total / max(B, 1))
